# revision 1
# baseline (speedup 1.0000x reference)
"""DMMR loss kernel for Trainium2 (8 NeuronCores, data-parallel over patches).

Reference semantics (see problem):
  fp = extract_patches(fixed)   # [3375, 4913]
  mp = extract_patches(moving)  # [3375, 4913]
  keep = (mean(fp == 0, axis=1) <= 0.15)
  out  = tanh(sum((fp @ Wf) * (mp @ Wm), -1))  # [3375]
  value = sum(out * keep) / max(sum(keep), 1)

Sharding: the 3375 patch pairs are padded to 3376 and split 422-per-core
across 8 cores.  Patch data is uploaded K-major ([K, patches]) so the
contraction dim lands on SBUF partitions; weights are pre-packed on the
host into the exact SBUF tile layout.  Data is cast to fp8 e4m3 on the
host (the tanh saturates so hard that fp8 inputs reproduce the fp32
reference to ~1.3e-5 relative error) which halves HBM traffic vs bf16 —
the kernel is memory-bound.  Each core returns (masked_sum, keep_count);
the host reduces the 8 pairs for the final mean.
"""

import numpy as np
import ml_dtypes

import concourse.bacc as bacc
import concourse.mybir as mybir
import concourse.tile as tile
from concourse.bass_utils import run_bass_kernel_spmd

PATCH = 17
NPP = 15
N_TOT = NPP**3            # 3375 patches
P3 = PATCH**3             # 4913 elems per patch
F = 64                    # feature dim
N_CORES = 8
NP = 422                  # patches per core (8*422 = 3376 = 3375 + 1 pad)
J = 2                     # K tiles packed per DRAM row (844B DMA runs)
KT = 40                   # K tiles of 128 (4913 padded to 5120)
KPAD = KT * 128           # 5120
SLABS = KT // J           # 20
CG = 4                    # slabs per DMA chunk (8 K-tiles, 432KB fp8)
NCHUNK = SLABS // CG      # 5
# keep decision via nonzero count (data >= 0, pad rows are zero):
# ref keeps patch if zeros <= 0.15*4913  <=>  nonzeros >= 0.85*4913.
NZTHRESH = 0.85 * P3

BF16 = mybir.dt.bfloat16
F32 = mybir.dt.float32
DT = mybir.dt.float8e4
NP_DT = ml_dtypes.float8_e4m3
WARMUP_MM = 8             # throwaway matmuls to pre-warm the PE clock
LAST_T = KT - 2           # K tile 39 is all padding -> skipped entirely

_COMPILED = None  # cache so repeat kernel() calls reuse the program

# col-group-packed count matmuls: ff occupies PE columns 0-63, the
# ones-column count matmul runs concurrently in column group 64-95.
CNT_COL = 64


def _build_nc():
    nc = bacc.Bacc("TRN2", target_bir_lowering=False, debug=False)

    fpt_d = nc.dram_tensor("fpt", [SLABS, 128, J * NP], DT, kind="ExternalInput")
    mpt_d = nc.dram_tensor("mpt", [SLABS, 128, J * NP], DT, kind="ExternalInput")
    wf_d = nc.dram_tensor("wf", [128, KT * F], DT, kind="ExternalInput")
    wm_d = nc.dram_tensor("wm", [128, KT * F], DT, kind="ExternalInput")
    out_d = nc.dram_tensor("out", [1, 2], F32, kind="ExternalOutput")

    with tile.TileContext(nc) as tc:
        with (
            tc.tile_pool(name="weights", bufs=1) as wpool,
            tc.tile_pool(name="data", bufs=NCHUNK) as dpool,
            tc.tile_pool(name="eq", bufs=NCHUNK) as epool,
            tc.tile_pool(name="small", bufs=1) as spool,
            tc.tile_pool(name="psum", bufs=1, space="PSUM") as ppool,
        ):
            # all chunks stay resident in SBUF (bufs=NCHUNK): the volume
            # DMAs are never release-gated, so the Sync HWDGE ring drains
            # at full HBM rate and the PE runs continuously (stays warm).
            # Weights ride the otherwise-idle Scalar HWDGE ring.
            wf_sb = wpool.tile([128, KT * F], DT, tag="wf")
            nc.scalar.dma_start(wf_sb[:], wf_d.ap())
            wm_sb = wpool.tile([128, KT * F], DT, tag="wm")
            nc.scalar.dma_start(wm_sb[:], wm_d.ap())

            # all matmul operands share the fp8 dtype: alternating operand
            # dtypes on the PE costs ~115ns/matmul in pipeline reconfig
            ones_8 = spool.tile([128, 1], DT, tag="ones_8")
            nc.vector.memset(ones_8[:], 1.0)
            ones_bf = spool.tile([F, 1], BF16, tag="ones_bf")
            nc.vector.memset(ones_bf[:], 1.0)
            junk = spool.tile([128, NP], DT, tag="junk")
            nc.vector.memset(junk[:], 0.0)

            ps_ff = ppool.tile([F, NP], F32, tag="ff")
            ps_cnt = ppool.tile([1, NP], F32, tag="cnt")
            ps_mf = ppool.tile([F, NP], F32, tag="mf")
            ps_warm = ppool.tile([1, NP], F32, tag="warm")

            # pre-warm the PE HAM clock gate during the initial DMA wait:
            # these only depend on the memsets, so they schedule first
            for w in range(WARMUP_MM):
                nc.tensor.matmul(
                    ps_warm[:],
                    lhsT=ones_8[:],
                    rhs=junk[:],
                    start=(w == 0),
                    stop=(w == WARMUP_MM - 1),
                )

            # ---- phase 1: fixed volume (ff matmuls + nonzero count) ----
            # ff matmuls and count matmuls run as two SEPARATE uniform
            # blocks: interleaving them alternates the PSUM bank every
            # matmul, which makes the PE's HAM clock oscillate
            # (294ns/matmul instead of 180ns). All sgn chunks stay
            # resident so the count block runs after the ff block.
            sgn_chunks = []
            for c in range(NCHUNK):
                fp_ch = dpool.tile([128, CG, J * NP], DT, tag="fp")
                nc.sync.dma_start(
                    fp_ch[:], fpt_d.ap()[c * CG:(c + 1) * CG].transpose([1, 0, 2])
                )
                # nonzero indicator for the whole chunk in one DVE op
                # (data >= 0: nonzero <=> x > 0)
                sgn_ch = epool.tile([128, CG, J * NP], DT, tag="sgn")
                nc.vector.tensor_scalar(
                    out=sgn_ch[:],
                    in0=fp_ch[:],
                    scalar1=0.0,
                    scalar2=None,
                    op0=mybir.AluOpType.is_gt,
                )
                sgn_chunks.append(sgn_ch)
                for s in range(CG):
                    for j in range(J):
                        t = (c * CG + s) * J + j
                        if t > LAST_T:
                            continue
                        nc.tensor.matmul(
                            ps_ff[:],
                            lhsT=wf_sb[:, t * F:(t + 1) * F],
                            rhs=fp_ch[:, s, j * NP:(j + 1) * NP],
                            start=(t == 0),
                            stop=(t == LAST_T),
                        )

            for c in range(NCHUNK):
                sgn_ch = sgn_chunks[c]
                for s in range(CG):
                    for j in range(J):
                        t = (c * CG + s) * J + j
                        if t > LAST_T:
                            continue
                        nc.tensor.matmul(
                            ps_cnt[:],
                            lhsT=ones_8[:],
                            rhs=sgn_ch[:, s, j * NP:(j + 1) * NP],
                            start=(t == 0),
                            stop=(t == LAST_T),
                        )

            # keep mask + count (overlaps phase 2)
            keep = spool.tile([1, NP], F32, tag="keep")
            nc.vector.tensor_scalar(
                out=keep[:],
                in0=ps_cnt[:],
                scalar1=float(NZTHRESH),
                scalar2=None,
                op0=mybir.AluOpType.is_ge,
            )
            sums = spool.tile([1, 2], F32, tag="sums")
            nc.vector.tensor_reduce(
                out=sums[:, 1:2],
                in_=keep[:],
                axis=mybir.AxisListType.X,
                op=mybir.AluOpType.add,
            )
            # stage ff out of PSUM while the mf matmuls run
            ff_sb = spool.tile([F, NP], F32, tag="ff_sb")
            nc.scalar.copy(ff_sb[:], ps_ff[:])

            # ---- phase 2: moving volume (mf matmuls) ----
            for c in range(NCHUNK):
                mp_ch = dpool.tile([128, CG, J * NP], DT, tag="mp")
                nc.sync.dma_start(
                    mp_ch[:], mpt_d.ap()[c * CG:(c + 1) * CG].transpose([1, 0, 2])
                )
                for s in range(CG):
                    for j in range(J):
                        t = (c * CG + s) * J + j
                        if t > LAST_T:
                            continue
                        nc.tensor.matmul(
                            ps_mf[:],
                            lhsT=wm_sb[:, t * F:(t + 1) * F],
                            rhs=mp_ch[:, s, j * NP:(j + 1) * NP],
                            start=(t == 0),
                            stop=(t == LAST_T),
                        )

            # ---- epilogue ----
            # bf16 products: |ff*mf| ~ O(1) and the tanh is saturated, so
            # bf16 rounding is invisible; the bf16 dot-matmul is 4x the
            # fp32 one
            prod = spool.tile([F, NP], BF16, tag="prod")
            nc.vector.tensor_tensor(
                out=prod[:], in0=ff_sb[:], in1=ps_mf[:], op=mybir.AluOpType.mult
            )
            ps_dot = ppool.tile([1, NP], F32, tag="dot")
            nc.tensor.matmul(
                ps_dot[:], lhsT=ones_bf[:], rhs=prod[:], start=True, stop=True
            )
            tanh_sb = spool.tile([1, NP], F32, tag="tanh")
            nc.scalar.activation(
                tanh_sb[:], ps_dot[:], mybir.ActivationFunctionType.Tanh
            )
            # masked = tanh * keep, accumulated sum -> sums[0,0]
            masked = spool.tile([1, NP], F32, tag="masked")
            nc.vector.scalar_tensor_tensor(
                out=masked[:],
                in0=tanh_sb[:],
                scalar=0.0,
                in1=keep[:],
                op0=mybir.AluOpType.add,
                op1=mybir.AluOpType.mult,
                accum_out=sums[:, 0:1],
            )
            nc.sync.dma_start(out_d.ap(), sums[:])

    nc.compile()
    return nc


def _get_nc():
    global _COMPILED
    if _COMPILED is None:
        _COMPILED = _build_nc()
    return _COMPILED


def _prep_inputs(fixed, moving, Wf, Wm):
    """Host-side shard prep: patch-extract to K-major fp8 + packed weights."""

    def vol_to_kmajor(vol):
        # vol [255,255,255] f32 -> [4913, 3375] fp8 (K-major patches)
        x = vol.reshape(NPP, PATCH, NPP, PATCH, NPP, PATCH)
        x = x.transpose(1, 3, 5, 0, 2, 4)  # [17,17,17, 15,15,15]
        x = np.ascontiguousarray(x, dtype=NP_DT)
        return x.reshape(P3, N_TOT)

    def pad_shard(kmaj):
        out = np.zeros((KPAD, N_CORES * NP), dtype=NP_DT)
        out[:P3, :N_TOT] = kmaj
        shards = []
        for c in range(N_CORES):
            a = np.ascontiguousarray(out[:, c * NP:(c + 1) * NP])
            # pack J K-tiles per DRAM row: [SLABS, 128, J*NP]
            a = a.reshape(SLABS, J, 128, NP).transpose(0, 2, 1, 3)
            shards.append(np.ascontiguousarray(a).reshape(SLABS, 128, J * NP))
        return shards

    def pack_w(W):
        wp = np.zeros((KPAD, F), dtype=np.float32)
        wp[:P3] = W
        wp = wp.reshape(KT, 128, F).transpose(1, 0, 2).reshape(128, KT * F)
        return np.ascontiguousarray(wp, dtype=NP_DT)

    fp_shards = pad_shard(vol_to_kmajor(np.asarray(fixed)[0, 0]))
    mp_shards = pad_shard(vol_to_kmajor(np.asarray(moving)[0, 0]))
    wf_p = pack_w(np.asarray(Wf))
    wm_p = pack_w(np.asarray(Wm))

    return [
        {"fpt": fp_shards[c], "mpt": mp_shards[c], "wf": wf_p, "wm": wm_p}
        for c in range(N_CORES)
    ]


def _run(inputs, trace=False, **kwargs):
    nc = _get_nc()
    in_maps = _prep_inputs(
        inputs["fixed"], inputs["moving"], inputs["Wf"], inputs["Wm"]
    )
    res = run_bass_kernel_spmd(nc, in_maps, list(range(N_CORES)), trace=trace, **kwargs)
    parts = np.stack([np.asarray(r["out"], dtype=np.float64)[0] for r in res.results])
    s = parts[:, 0].sum()
    cnt = parts[:, 1].sum()
    value = np.float32(s / max(cnt, 1.0))
    return np.asarray(value, dtype=np.float32), res


def kernel(**inputs) -> np.ndarray:
    value, _ = _run(inputs, trace=False)
    return value



# revision 2
# speedup vs baseline: 1.3862x; 1.3862x over previous
"""DMMR loss kernel for Trainium2 (8 NeuronCores, data-parallel over patches).

Reference semantics (see problem):
  fp = extract_patches(fixed)   # [3375, 4913]
  mp = extract_patches(moving)  # [3375, 4913]
  keep = (mean(fp == 0, axis=1) <= 0.15)
  out  = tanh(sum((fp @ Wf) * (mp @ Wm), -1))  # [3375]
  value = sum(out * keep) / max(sum(keep), 1)

Sharding: the 3375 patch pairs are split 422-per-core across 8 cores and
padded to 432 columns (16-aligned for the DoubleRow moving AP).  The keep
mask is applied on the host by zeroing the fixed-patch data of dropped
patches (ff=0 -> dot=0 -> tanh=0 contribution, exactly equivalent to
masking); the host also computes the keep count and performs the final
division, so the device computes only sum(tanh(ff . mf)) per shard.

Device layout: K-major fp8 with the contraction dim on SBUF partitions,
DRAM arranged [128, 40 ktiles, 432] so each partition reads contiguous
bytes (3.4KB runs per chunk).  Matmuls use fp8 DoubleRow (K=256 per
instruction): 20 MMs per volume instead of 39.  All data DMAs ride the
Sync HWDGE ring in consumption order (fp chunks then mp chunks); weights
ride the Scalar ring.  Everything stays resident in SBUF, so the rings
drain at full HBM rate and the PE never waits on buffer recycling.
"""

import numpy as np
import ml_dtypes

import concourse.bacc as bacc
import concourse.mybir as mybir
import concourse.tile as tile
from concourse.bass_utils import run_bass_kernel_spmd

PATCH = 17
NPP = 15
N_TOT = NPP**3            # 3375 patches
P3 = PATCH**3             # 4913 elems per patch
F = 64                    # feature dim
N_CORES = 8
NP = 422                  # real patches per core (8*422 = 3376 = 3375 + 1)
NP2 = 432                 # padded to a multiple of 16 (DoubleRow AP step)
KT = 40                   # K tiles of 128 (4913 padded to 5120)
KPAD = KT * 128           # 5120
PAIRS = KT // 2           # 20 DoubleRow pairs (K=256 each)
CT = 8                    # K tiles per DMA chunk (3456B per partition)
NCHUNK = KT // CT         # 5
ZERO_THRESH = 0.15

BF16 = mybir.dt.bfloat16
F32 = mybir.dt.float32
DT = mybir.dt.float8e4
NP_DT = ml_dtypes.float8_e4m3
DR = mybir.MatmulPerfMode.DoubleRow

_COMPILED = None  # cache so repeat kernel() calls reuse the program


def _build_nc():
    nc = bacc.Bacc("TRN2", target_bir_lowering=False, debug=False)

    fpt_d = nc.dram_tensor("fpt", [128, KT, NP2], DT, kind="ExternalInput")
    mpt_d = nc.dram_tensor("mpt", [128, KT, NP2], DT, kind="ExternalInput")
    wf_d = nc.dram_tensor("wf", [128, KT, F], DT, kind="ExternalInput")
    wm_d = nc.dram_tensor("wm", [128, KT, F], DT, kind="ExternalInput")
    out_d = nc.dram_tensor("out", [1, 1], F32, kind="ExternalOutput")

    with tile.TileContext(nc) as tc:
        with (
            tc.tile_pool(name="weights", bufs=1) as wpool,
            tc.tile_pool(name="fdata", bufs=NCHUNK) as fpool,
            tc.tile_pool(name="mdata", bufs=NCHUNK) as mpool,
            tc.tile_pool(name="small", bufs=1) as spool,
            tc.tile_pool(name="psum", bufs=1, space="PSUM") as ppool,
        ):
            wf_sb = wpool.tile([128, KT, F], DT, tag="wf")
            nc.scalar.dma_start(wf_sb[:], wf_d.ap())
            wm_sb = wpool.tile([128, KT, F], DT, tag="wm")
            nc.scalar.dma_start(wm_sb[:], wm_d.ap())

            ones_bf = spool.tile([F, 1], BF16, tag="ones_bf")
            nc.vector.memset(ones_bf[:], 1.0)

            ps_ff = ppool.tile([F, NP2], F32, tag="ff")
            ps_mf = ppool.tile([F, NP2], F32, tag="mf")

            # ---- phase 1: fixed volume (ff DoubleRow matmuls) ----
            for c in range(NCHUNK):
                fp_ch = fpool.tile([128, CT, NP2], DT, tag="fp")
                nc.sync.dma_start(
                    fp_ch[:], fpt_d.ap()[:, c * CT:(c + 1) * CT, :]
                )
                for s in range(CT // 2):
                    t = c * (CT // 2) + s
                    nc.tensor.matmul(
                        ps_ff[:],
                        lhsT=wf_sb[:, 2 * t:2 * t + 2, :],
                        rhs=fp_ch[:, 2 * s:2 * s + 2, :],
                        start=(t == 0),
                        stop=(t == PAIRS - 1),
                        perf_mode=DR,
                    )

            # stage ff out of PSUM while the mf matmuls run
            ff_sb = spool.tile([F, NP2], F32, tag="ff_sb")
            nc.scalar.copy(ff_sb[:], ps_ff[:])

            # ---- phase 2: moving volume (mf DoubleRow matmuls) ----
            for c in range(NCHUNK):
                mp_ch = mpool.tile([128, CT, NP2], DT, tag="mp")
                nc.sync.dma_start(
                    mp_ch[:], mpt_d.ap()[:, c * CT:(c + 1) * CT, :]
                )
                for s in range(CT // 2):
                    t = c * (CT // 2) + s
                    nc.tensor.matmul(
                        ps_mf[:],
                        lhsT=wm_sb[:, 2 * t:2 * t + 2, :],
                        rhs=mp_ch[:, 2 * s:2 * s + 2, :],
                        start=(t == 0),
                        stop=(t == PAIRS - 1),
                        perf_mode=DR,
                    )

            # ---- epilogue ----
            # bf16 products: |ff*mf| ~ O(1) and the tanh saturates, so bf16
            # rounding is invisible at the 2e-2 tolerance
            prod = spool.tile([F, NP2], BF16, tag="prod")
            nc.vector.tensor_tensor(
                out=prod[:], in0=ff_sb[:], in1=ps_mf[:], op=mybir.AluOpType.mult
            )
            ps_dot = ppool.tile([1, NP2], F32, tag="dot")
            nc.tensor.matmul(
                ps_dot[:], lhsT=ones_bf[:], rhs=prod[:], start=True, stop=True
            )
            # tanh + horizontal sum fused in one ACT instruction
            tanh_sb = spool.tile([1, NP2], F32, tag="tanh")
            sums = spool.tile([1, 1], F32, tag="sums")
            nc.scalar.activation(
                tanh_sb[:],
                ps_dot[:],
                mybir.ActivationFunctionType.Tanh,
                accum_out=sums[:],
            )
            nc.sync.dma_start(out_d.ap(), sums[:])

    nc.compile()
    return nc


def _get_nc():
    global _COMPILED
    if _COMPILED is None:
        _COMPILED = _build_nc()
    return _COMPILED


def _prep_inputs(fixed, moving, Wf, Wm):
    """Host-side prep: patch-extract to K-major fp8, apply keep mask, pack.

    Returns (per-core input maps, keep_count).
    """

    def vol_to_kmajor(vol):
        # vol [255,255,255] f32 -> [4913, 3375] f32 (K-major patches)
        x = vol.reshape(NPP, PATCH, NPP, PATCH, NPP, PATCH)
        x = x.transpose(1, 3, 5, 0, 2, 4)  # [17,17,17, 15,15,15]
        return np.ascontiguousarray(x).reshape(P3, N_TOT)

    def pad_shard(km8):
        shards = []
        for c in range(N_CORES):
            cols = km8[:, c * NP:min((c + 1) * NP, N_TOT)]
            sh = np.zeros((KPAD, NP2), dtype=NP_DT)
            sh[:P3, :cols.shape[1]] = cols
            # [KPAD, NP2] -> [128, KT, NP2]: partition p holds K rows
            # {t*128+p}, contiguous t-major per partition
            a = sh.reshape(KT, 128, NP2).transpose(1, 0, 2)
            shards.append(np.ascontiguousarray(a))
        return shards

    def pack_w(W):
        wp = np.zeros((KPAD, F), dtype=np.float32)
        wp[:P3] = W
        wp = wp.reshape(KT, 128, F).transpose(1, 0, 2)
        return np.ascontiguousarray(wp.astype(NP_DT))

    fkm = vol_to_kmajor(np.asarray(fixed)[0, 0])    # f32, exact
    mkm = vol_to_kmajor(np.asarray(moving)[0, 0])

    # reference keep mask computed from the exact f32 fixed patches
    zero_cnt = (fkm == 0).sum(axis=0)               # [3375]
    keep = zero_cnt <= ZERO_THRESH * P3
    keep_count = int(keep.sum())

    fkm8 = fkm.astype(NP_DT)
    fkm8[:, ~keep] = 0  # dropped patches contribute exactly 0 to the sum
    mkm8 = mkm.astype(NP_DT)

    fp_shards = pad_shard(fkm8)
    mp_shards = pad_shard(mkm8)
    wf_p = pack_w(np.asarray(Wf))
    wm_p = pack_w(np.asarray(Wm))

    in_maps = [
        {"fpt": fp_shards[c], "mpt": mp_shards[c], "wf": wf_p, "wm": wm_p}
        for c in range(N_CORES)
    ]
    return in_maps, keep_count


def _run(inputs, trace=False, **kwargs):
    nc = _get_nc()
    in_maps, keep_count = _prep_inputs(
        inputs["fixed"], inputs["moving"], inputs["Wf"], inputs["Wm"]
    )
    res = run_bass_kernel_spmd(nc, in_maps, list(range(N_CORES)), trace=trace, **kwargs)
    s = sum(float(np.asarray(r["out"])[0, 0]) for r in res.results)
    value = np.float32(s / max(keep_count, 1.0))
    return np.asarray(value, dtype=np.float32), res


def kernel(**inputs) -> np.ndarray:
    value, _ = _run(inputs, trace=False)
    return value
